# revision 27
# baseline (speedup 1.0000x reference)
"""Trainium2 Bass kernel for nn_ColorTransform: per-pixel degree-3 polynomial
color transform  y[b,c,h,w] = bias[c] + sum_f weight[f,c] * mono_f(x[b,:,h,w]).

Baseline pipeline skeleton (proven overlap) with an R=7 shifted-square basis:

    u_i = a_i . x + b_i            (PE, M1, fp16)
    S_i = (u_i + d_i)^2            (ACT square with per-partition bias d_i)
    Q_i = S_i * u_i                (DVE elementwise multiply)
    y_c = sum_i cq[i,c] Q_i + cs[i,c] S_i     (PE, M2, fp32r, PSUM accum)

R=7 forms suffice (7*(a3+b1+d1+cq3+cs3-1gauge)=70 >= 60 target coeffs), so
18 pixels pack per partition column (vs 12 for the R=10 cubes+squares basis),
cutting every engine's per-pass cost by a third. Forms are solved on the host
(Gauss-Newton from a precomputed init; weight/bias are tiny), a,b quantized to
fp16, then d and the coefficients re-polished.
"""
import numpy as np

import concourse.bass as bass
import concourse.tile as tile
from concourse import bacc, mybir
from concourse.bass_utils import run_bass_kernel_spmd

# ---------------------------------------------------------------- constants
B, C, H, W = 16, 3, 512, 512
HW = H * W
NCORES = 8
BPC = B // NCORES          # batches per core = 2
R = 7                      # affine forms per group
GPB = (126 // R) // BPC    # groups per batch per chunk = 9
NG = BPC * GPB
ND = 8192                  # DMA columns per chunk
NCMP = 1024                # compute columns per sub-chunk
PS_BUFS = 2
P2_BUFS = 2
SQ_DEPTH = 1
SPLIT = ND // NCMP
FULL_CHUNKS = HW // (GPB * ND)            # 3
TAIL_PX = HW - FULL_CHUNKS * GPB * ND     # 40960
TAIL_GPB = TAIL_PX // ND                  # 5
ACT_FRAC = 1.0

MONOMIALS = [
    (1,0,0),(0,1,0),(0,0,1),
    (2,0,0),(1,1,0),(1,0,1),(0,2,0),(0,1,1),(0,0,2),
    (3,0,0),(2,1,0),(2,0,1),(1,2,0),(1,1,1),(1,0,2),(0,3,0),(0,2,1),(0,1,2),(0,0,3),
]

THETA7 = np.array([
    [ -0.71232073,   0.55435081,  -0.4304583 , -10.98574337,  11.50086715],
    [  0.12142046,  -0.75639196,  -0.64275056,   0.68928347,   0.25387477],
    [  0.04290576,  -0.04260192,  -0.99817041,   1.11254654,  -0.83063126],
    [  0.79598004,   0.60314031,  -0.05135698,  -1.63278377,   0.70114269],
    [  0.02904839,   0.67443339,  -0.73776405,   1.02076177,  -1.1416206 ],
    [  0.28919363,   0.91790785,   0.27168406,  -5.43342412,   6.01042838],
    [  0.86276392,   0.50191628,  -0.06097928,  -2.58808322,   1.75123132]])


# ---------------------------------------------------------------- host solve
def _target_eval(X, weight, bias):
    feats = np.stack([X[:,0]**m[0] * X[:,1]**m[1] * X[:,2]**m[2]
                      for m in MONOMIALS], 1)
    return feats @ np.asarray(weight, np.float64) + np.asarray(bias, np.float64)[None, :]


def _basis_eval(theta, X):
    a = theta[:, :3]; b = theta[:, 3]; d = theta[:, 4]
    U = X @ a.T + b[None, :]
    S = (U + d[None, :]) ** 2
    return np.concatenate([S * U, S], axis=1)


def _gn(theta0, Xs, T, free_mask, renorm, n_iter=60, tol=1e-13):
    theta = theta0.copy()
    idx = np.where(free_mask.ravel())[0]

    def norm(th):
        if not renorm:
            return th
        th = th.copy()
        n = np.linalg.norm(th[:, :3], axis=1, keepdims=True)
        th[:, :3] /= n
        th[:, 3:5] /= n
        return th

    def resid(th):
        th = norm(th)
        Bm = _basis_eval(th, Xs)
        Cc, *_ = np.linalg.lstsq(Bm, T, rcond=None)
        return (Bm @ Cc - T).ravel(), Cc

    r, Cc = resid(theta)
    lam = 1e-7
    for _ in range(n_iter):
        J = np.empty((r.size, idx.size))
        h = 1e-7
        for j, k in enumerate(idx):
            f2 = theta.ravel().copy(); f2[k] += h
            r2, _ = resid(f2.reshape(theta.shape))
            J[:, j] = (r2 - r) / h
        ok = False
        for _ in range(8):
            A = J.T @ J + lam * np.eye(idx.size)
            step = np.linalg.solve(A, J.T @ r)
            f2 = theta.ravel().copy(); f2[idx] -= step
            r2, C2 = resid(f2.reshape(theta.shape))
            if (r2 @ r2) < (r @ r):
                theta = f2.reshape(theta.shape); r, Cc = r2, C2
                lam = max(lam * 0.3, 1e-12); ok = True
                break
            lam *= 10
        if not ok or r @ r < tol:
            break
    return norm(theta), Cc, float(np.abs(r).max())


_SOLVE_CACHE = {}


def _solve(weight, bias):
    """-> (theta[R,5] with a,b fp16-exact, Cq [R,3], Cs [R,3])"""
    key = (np.asarray(weight).tobytes(), np.asarray(bias).tobytes())
    if key in _SOLVE_CACHE:
        return _SOLVE_CACHE[key]
    rng = np.random.default_rng(12345)
    Xs = rng.uniform(-0.25, 1.25, size=(160, 3))
    T = _target_eval(Xs, weight, bias)

    th, _, res = _gn(THETA7.copy(), Xs, T, np.ones((R, 5), bool), renorm=True)
    if res > 1e-6:
        for _ in range(8):
            th0 = np.concatenate([rng.normal(size=(R, 3)),
                                  rng.normal(size=(R, 2)) * 0.7], 1)
            th, _, res = _gn(th0, Xs, T, np.ones((R, 5), bool), renorm=True)
            if res < 1e-6:
                break
    # quantize a,b to fp16 (they ride in the fp16 M1 weights); re-polish d + C
    th[:, :4] = th[:, :4].astype(np.float16).astype(np.float64)
    mask = np.zeros((R, 5), bool); mask[:, 4] = True
    th, Cc, _ = _gn(th, Xs, T, mask, renorm=False, n_iter=30)
    out = (th, Cc[:R], Cc[R:])
    _SOLVE_CACHE[key] = out
    return out


# v-major row maps (baseline layout) ----------------------------------------
# X rows: 0 = ones; 1 + b*(3*gpb) + v*gpb + g
# P1 rows: b*(R*gpb) + i*gpb + g
# P2/O rows: b*(3*gpb) + c*gpb + g

def _lhs1(av, bv, gpb):
    KX = 3 * gpb * BPC + 1
    m = np.zeros((KX, R * gpb * BPC), np.float32)
    for b in range(BPC):
        for g in range(gpb):
            for i in range(R):
                col = b*R*gpb + i*gpb + g
                m[0, col] = bv[i]
                for v in range(C):
                    m[1 + b*3*gpb + v*gpb + g, col] = av[i, v]
    return m.astype(np.float16)


def _lhs2(coeff, gpb):
    m = np.zeros((R * gpb * BPC, 3 * gpb * BPC), np.float32)
    for b in range(BPC):
        for g in range(gpb):
            for i in range(R):
                for c in range(C):
                    m[b*R*gpb + i*gpb + g, b*3*gpb + c*gpb + g] = coeff[i, c]
    return m


def _dltmap(dv, gpb):
    m = np.zeros((R * gpb * BPC, 16), np.float32)
    for b in range(BPC):
        for g in range(gpb):
            for i in range(R):
                m[b*R*gpb + i*gpb + g, :] = dv[i]
    return m


# ---------------------------------------------------------------- bass build
_NC_CACHE = {}


def build_nc(reps=1, chunks=None):
    key = (reps, chunks)
    if key in _NC_CACHE:
        return _NC_CACHE[key]
    f32, f16, f32r = mybir.dt.float32, mybir.dt.float16, mybir.dt.float32r
    AFT = mybir.ActivationFunctionType
    nc = bacc.Bacc("TRN2", target_bir_lowering=False, debug=False, num_devices=NCORES)

    KX = 3 * GPB * BPC + 1
    RW = R * GPB * BPC
    OW = 3 * GPB * BPC
    KXt = 3 * TAIL_GPB * BPC + 1
    RWt = R * TAIL_GPB * BPC
    OWt = 3 * TAIL_GPB * BPC

    xs = nc.dram_tensor("xs", [BPC, C, HW], f16, kind="ExternalInput")
    wm1 = nc.dram_tensor("wm1", [KX, RW], f16, kind="ExternalInput")
    w2q = nc.dram_tensor("w2q", [RW, OW], f32, kind="ExternalInput")
    w2s = nc.dram_tensor("w2s", [RW, OW], f32, kind="ExternalInput")
    dlt = nc.dram_tensor("dlt", [RW, 16], f32, kind="ExternalInput")
    y = nc.dram_tensor("y", [BPC, C, HW], f32, kind="ExternalOutput")
    if TAIL_GPB:
        wm1t = nc.dram_tensor("wm1t", [KXt, RWt], f16, kind="ExternalInput")
        w2qt = nc.dram_tensor("w2qt", [RWt, OWt], f32, kind="ExternalInput")
        w2st = nc.dram_tensor("w2st", [RWt, OWt], f32, kind="ExternalInput")
        dltt = nc.dram_tensor("dltt", [RWt, 16], f32, kind="ExternalInput")

    with tile.TileContext(nc) as tc:
        with (
            tc.tile_pool(name="wpool", bufs=1) as wpool,
            tc.tile_pool(name="xpool", bufs=2) as xpool,
            tc.tile_pool(name="spool", bufs=2) as spool,
            tc.tile_pool(name="qpool", bufs=2) as qpool,
            tc.tile_pool(name="opool", bufs=2) as opool,
            tc.tile_pool(name="p1pool", bufs=PS_BUFS, space="PSUM") as p1pool,
            tc.tile_pool(name="p2pool", bufs=P2_BUFS, space="PSUM") as p2pool,
        ):
            def load_w(name, dram, shape, dt_, round_to=None):
                t = wpool.tile(shape, dt_, tag=name)
                nc.sync.dma_start(t[:], dram[:])
                if round_to is None:
                    return t
                tr = wpool.tile(shape, round_to, tag=name + "r")
                nc.vector.tensor_copy(tr[:], t[:])
                return tr

            wm1_sb = load_w("wm1", wm1, [KX, RW], f16)
            w2q_r = load_w("w2q", w2q, [RW, OW], f32, f32r)
            w2s_r = load_w("w2s", w2s, [RW, OW], f32, f32r)
            dlt_sb = load_w("dlt", dlt, [RW, 16], f32)
            if TAIL_GPB:
                wm1t_sb = load_w("wm1t", wm1t, [KXt, RWt], f16)
                w2qt_r = load_w("w2qt", w2qt, [RWt, OWt], f32, f32r)
                w2st_r = load_w("w2st", w2st, [RWt, OWt], f32, f32r)
                dltt_sb = load_w("dltt", dltt, [RWt, 16], f32)

            for _ in range(2):
                xt0 = xpool.tile([KX, ND], f16, tag="X")
                nc.gpsimd.memset(xt0[0:1, :], 1.0)

            asp = (int(NCMP * ACT_FRAC) // 2) * 2

            def stage1(lo, gpb, m1w, tagsuf, q, xt_state):
                kx = 3 * gpb * BPC + 1
                rw = R * gpb * BPC
                if q == 0:
                    xt = xpool.tile([kx, ND], f16, tag="X", name="xt" + tagsuf)
                    if tagsuf:
                        nc.gpsimd.memset(xt[0:1, :], 1.0)
                    for b in range(BPC):
                        nc.sync.dma_start(
                            xt[1+b*3*gpb:1+(b+1)*3*gpb],
                            xs[b, :, lo:lo+gpb*ND].rearrange("v (g n) -> v g n", n=ND))
                    xt_state[tagsuf] = xt
                xt = xt_state[tagsuf]
                cl = q * NCMP
                p1 = p1pool.tile([rw, NCMP], f32, tag="P1")
                for h in range(NCMP // 512):
                    nc.tensor.matmul(p1[:, h*512:(h+1)*512], m1w[:],
                                     xt[:, cl+h*512:cl+(h+1)*512],
                                     start=True, stop=True)
                return p1

            def stage2(gpb, dsb, p1):
                rw = R * gpb * BPC
                s = spool.tile([rw, NCMP], f32r, tag="S")
                nc.scalar.activation(s[:], p1[:], AFT.Square,
                                     bias=dsb[:, 0:1], scale=1.0)
                qq = qpool.tile([rw, NCMP], f32r, tag="Q")
                nc.vector.tensor_mul(qq[:], s[:], p1[:])
                return s, qq

            def stage3(lo, gpb, qw, sw, tagsuf, q, s, qq, o_state):
                rw = R * gpb * BPC
                ow = 3 * gpb * BPC
                if q == 0:
                    o_state[tagsuf] = opool.tile([ow, ND], f32, tag="O", name="o" + tagsuf)
                o = o_state[tagsuf]
                cl, ch = q * NCMP, (q + 1) * NCMP
                p2 = p2pool.tile([ow, NCMP], f32, tag="P2")
                for h in range(NCMP // 512):
                    hl, hh = h*512, (h+1)*512
                    nc.tensor.matmul(p2[:, hl:hh], qw[:], qq[:, hl:hh],
                                     start=True, stop=False)
                for h in range(NCMP // 512):
                    hl, hh = h*512, (h+1)*512
                    nc.tensor.matmul(p2[:, hl:hh], sw[:], s[:, hl:hh],
                                     start=False, stop=True)
                nc.scalar.copy(o[:, cl:cl+asp], p2[:, 0:asp])
                if asp < NCMP:
                    nc.vector.tensor_copy(o[:, cl+asp:ch], p2[:, asp:NCMP])
                if q == SPLIT - 1:
                    for b in range(BPC):
                        nc.sync.dma_start(
                            y[b, :, lo:lo+gpb*ND].rearrange("c (g n) -> c g n", n=ND),
                            o[b*3*gpb:(b+1)*3*gpb])

            def body():
                nfull = FULL_CHUNKS if chunks is None else chunks
                work = [(k * GPB * ND, GPB, wm1_sb, w2q_r, w2s_r, dlt_sb, "")
                        for k in range(nfull)]
                if TAIL_GPB and chunks is None:
                    work.append((FULL_CHUNKS * GPB * ND, TAIL_GPB,
                                 wm1t_sb, w2qt_r, w2st_r, dltt_sb, "T"))
                units = [(lo, gpb, m1w, qw, sw, dsb, tagsuf, q)
                         for (lo, gpb, m1w, qw, sw, dsb, tagsuf) in work
                         for q in range(SPLIT)]
                xt_state, o_state = {}, {}
                q2 = []
                q3 = []
                D2, D3 = SQ_DEPTH, 1

                def pump(force=False):
                    if q2 and (force or len(q2) > D2 - 1):
                        (l2, g2, q2w, s2w, d2, t2, qu2, p12) = q2.pop(0)
                        s_t, qq_t = stage2(g2, d2, p12)
                        q3.append((l2, g2, q2w, s2w, t2, qu2, s_t, qq_t, o_state))
                    if q3 and (force or len(q3) > D3 - 1):
                        stage3(*q3.pop(0))

                for (lo, gpb, m1w, qw, sw, dsb, tagsuf, q) in units:
                    p1 = stage1(lo, gpb, m1w, tagsuf, q, xt_state)
                    q2.append((lo, gpb, qw, sw, dsb, tagsuf, q, p1))
                    pump()
                while q2 or q3:
                    pump(force=True)

            if reps == 1:
                body()
            else:
                hint = (mybir.EngineType.PE, mybir.EngineType.Activation,
                        mybir.EngineType.DVE, mybir.EngineType.SP)
                with tc.For_i(0, reps, 1, hint_engines=hint):
                    body()

    nc.compile()
    _NC_CACHE[key] = nc
    return nc


def make_in_maps(x, weight, bias):
    theta, cq, cs = _solve(weight, bias)
    av, bv, dv = theta[:, :3], theta[:, 3], theta[:, 4]
    shared = {
        "wm1": _lhs1(av, bv, GPB),
        "w2q": _lhs2(cq, GPB), "w2s": _lhs2(cs, GPB),
        "dlt": _dltmap(dv, GPB),
    }
    if TAIL_GPB:
        shared.update({
            "wm1t": _lhs1(av, bv, TAIL_GPB),
            "w2qt": _lhs2(cq, TAIL_GPB), "w2st": _lhs2(cs, TAIL_GPB),
            "dltt": _dltmap(dv, TAIL_GPB),
        })
    x = np.ascontiguousarray(np.asarray(x, np.float16)).reshape(B, C, HW)
    return [dict(shared, xs=x[i*BPC:(i+1)*BPC]) for i in range(NCORES)]


def kernel(x, weight, bias, degree=3, **_unused):
    assert int(degree) == 3, "kernel specialized for degree=3"
    nc = build_nc(reps=1)
    in_maps = make_in_maps(x, weight, bias)
    res = run_bass_kernel_spmd(nc, in_maps, core_ids=list(range(NCORES)))
    out = np.empty((B, C, HW), np.float32)
    for i in range(NCORES):
        out[i*BPC:(i+1)*BPC] = res.results[i]["y"]
    return out.reshape(B, C, H, W)


if __name__ == "__main__":
    rng = np.random.default_rng(0)
    x = rng.uniform(0, 1, size=(B, C, H, W)).astype(np.float32)
    weight = rng.normal(size=(19, 3)).astype(np.float32)
    bias = rng.normal(size=(3,)).astype(np.float32)
    got = kernel(x, weight, bias, 3)
    print("ran; out shape", got.shape)
